# revision 1
# baseline (speedup 1.0000x reference)
"""BERT attention (QKV proj + SDPA) sharded over 8 trn2 NeuronCores by head.

Problem: hidden_states [2, 2048, 1024], 16 heads x 64 dim, fp32.
Sharding: 2 heads per core (tensor-parallel on Q/K/V weight columns).

Per-core device kernel (matmul operands fp16, accumulation fp32):
  inputs:  xt  [1024, 4096]  X^T (host-pretransposed, fp16, same on all cores)
           wq/wk/wv [1024, 128]  weight column slice for this core's 2 heads
           bias [128, 3]         q/k/v bias slices packed (f32)
  output:  out [4096, 128] f32   context for the 2 heads (token-major)

Dataflow per batch:
  1. QT/KT/VT [c=128, t] = W.T @ X.T (contraction over hidden), bias added
     on DVE during PSUM->SBUF copy (fp16 out).
  2. V' [k, 65] per head via PE-transpose of VT; col 64 = ones (row sums).
  3. Scores TRANSPOSED: ST[k, q] f32 psum; exp -> PT[k, q] fp16. exp runs on
     ACT (native Exp) for most tiles and on DVE for some via the Schraudolph
     u16-bits trick (bits = round(s*1024/(8 ln2) + 15301.5), bitcast fp16) to
     keep ACT off the critical path. The bits bias is centered so mixed
     engine flavors agree in the mean; softmax row sums normalize the rest.
  4. P@V with q STATIONARY: ctx[q=128, d|sum 65] = sum_kt PT[k,qc].T @ V'[k,65]
     -- the 65-wide moving operand makes each accumulation step cost 65
     columns instead of 512, halving PE time vs the moving-PT layout, and
     the output lands q-major so normalization needs no PE transposes:
     DVE reciprocal of the sums column + tensor_scalar multiply, DMA out.

The kt loop is PE-bound; projection matmuls, V' transposes and deferred
P@V chunks are emitted as fillers inside the kt loop paced by a credit
system. PSUM budget: scores 2x2 banks, proj/transpose ring 2x1, ctx 2x1.
"""

import numpy as np

B, S, HID = 2, 2048, 1024
T = B * S
N_CORES = 8
P = 128
D = 64
HK = HID // P  # hidden-dim chunks

F16 = np.float16

_CACHED = {}

# exp tiles routed to DVE (u16-bits trick): per-unit set of (kt, head).
# Unit 3 gets more: its filler queue is short, so the kt loop is paced by
# exp completions (st-ring WAR) and DVE relieves ACT.
_DVE_BASE = {(2, 0), (5, 1), (8, 0), (11, 1), (14, 0), (14, 1)}
DVE_EXP = {
    0: _DVE_BASE,
    1: _DVE_BASE,
    2: _DVE_BASE,
    3: _DVE_BASE | {(1, 1), (4, 0), (7, 1), (10, 0), (13, 1), (15, 0)},
}
A16 = 0.125 * 1024.0 / np.log(2.0)
B16 = 15.0 * 1024.0 - 58.5  # centered Schraudolph bias


def _build():
    from collections import deque

    import concourse.bass as bass
    from concourse import bacc
    import concourse.tile as tile
    import concourse.mybir as mybir
    from concourse.bass import ts, ds
    from concourse.masks import make_identity

    f16 = mybir.dt.float16
    f32 = mybir.dt.float32
    u16 = mybir.dt.uint16
    Exp = mybir.ActivationFunctionType.Exp

    nc = bacc.Bacc(trn_type="TRN2", target_bir_lowering=False, debug=False)

    xt = nc.dram_tensor("xt", [HID, T], f16, kind="ExternalInput").ap()
    wq = nc.dram_tensor("wq", [HID, P], f16, kind="ExternalInput").ap()
    wk = nc.dram_tensor("wk", [HID, P], f16, kind="ExternalInput").ap()
    wv = nc.dram_tensor("wv", [HID, P], f16, kind="ExternalInput").ap()
    bias = nc.dram_tensor("bias", [P, 3], f32, kind="ExternalInput").ap()
    out = nc.dram_tensor("out", [T, P], f32, kind="ExternalOutput").ap()

    with tile.TileContext(nc) as tc:
        with (
            tc.tile_pool(name="const", bufs=1) as cpool,
            tc.tile_pool(name="xtp", bufs=1) as xtpool,
            tc.tile_pool(name="qkv", bufs=1) as qkvpool,
            tc.tile_pool(name="pt", bufs=1) as ptpool,
            tc.tile_pool(name="small", bufs=4) as smallpool,
            tc.tile_pool(name="ot", bufs=3) as otpool,
            tc.tile_pool(name="ps", bufs=2, space="PSUM") as psp,
        ):
            # X^T half-buffer: one batch's tokens; batch 1 reloads it.
            xt_sb = xtpool.tile([P, HK, S], f16, tag="xt")
            xtp = xt.rearrange("(a p) t -> p a t", p=P)
            w_sbs = []
            bias_sb = cpool.tile([P, 3], f32, tag="bias")
            b_sbs = [bias_sb[:, i : i + 1] for i in range(3)]
            for i, name in enumerate(("q", "k", "v")):
                w_sbs.append(
                    cpool.tile([P, HK, P], f16, tag=f"w{name}", name=f"w{name}sb")
                )
            # DMA arrival order matched to first-consumption order
            nc.sync.dma_start(xt_sb[:, :, 0:512], xtp[:, :, 0:512])
            nc.sync.dma_start(bias_sb, bias)
            nc.sync.dma_start(w_sbs[0], wq.rearrange("(a p) c -> p a c", p=P))
            nc.sync.dma_start(xt_sb[:, :, ts(1, 512)], xtp[:, :, ts(1, 512)])
            nc.sync.dma_start(w_sbs[1], wk.rearrange("(a p) c -> p a c", p=P))
            nc.sync.dma_start(w_sbs[2], wv.rearrange("(a p) c -> p a c", p=P))
            for quarter in range(2, 4):
                nc.sync.dma_start(
                    xt_sb[:, :, ts(quarter, 512)], xtp[:, :, ts(quarter, 512)]
                )

            ident = cpool.tile([P, P], f16, tag="ident")
            make_identity(nc, ident)
            # zero operand for PE warm-up (values unused; Pool engine is idle)
            garb = cpool.tile([P, 512], f16, tag="garb")
            nc.gpsimd.memset(garb, 0.0)

            qt_sb = qkvpool.tile([P, T], f16, tag="qt")
            kt_sb = qkvpool.tile([P, T], f16, tag="kt")
            vt_sb = qkvpool.tile([P, T], f16, tag="vt")
            # V' per head: [k-part, ktile, 65]; col 64 = ones for row sums
            vp_sb = qkvpool.tile([P, 2, T // P, D + 1], f16, tag="vp")
            nc.vector.memset(vp_sb[:, :, :, D : D + 1], 1.0)

            # PE warm-up while the first DMAs land: long enough that the
            # p-state ramp completes AND the PE never idles before the first
            # projection (idle would reset the ramp and run projs at 2-4x).
            wu = psp.tile([P, 512], f32, tag="pj", bufs=2, name="wups")
            NWU = 17
            for i in range(NWU):
                nc.tensor.matmul(wu, ident, garb, start=(i == 0), stop=(i == NWU - 1))
            nc.vector.tensor_copy(vp_sb[:, 0, 0, 0:D], wu[:, 0:D])

            def proj_group(t8, which):
                """Project 512 tokens (chunk t8) for q/k/v (which=0/1/2)."""
                w_sb, b_sb = w_sbs[which], b_sbs[which]
                dst = (qt_sb, kt_sb, vt_sb)[which]
                ps = psp.tile([P, 512], f32, tag="pj", bufs=2, name="projps")
                for a in range(HK):
                    nc.tensor.matmul(
                        ps,
                        w_sb[:, a, :],
                        xt_sb[:, a, ts(t8 % 4, 512)],
                        start=(a == 0),
                        stop=(a == HK - 1),
                    )
                nc.vector.tensor_scalar_add(dst[:, ts(t8, 512)], ps, b_sb)

            def vprime(head, kt32):
                """Transpose one [64,128] VT tile into V'[:, head, kt32]."""
                tp = psp.tile([P, D], f16, tag="pj", bufs=2, name="vtps")
                nc.tensor.transpose(
                    tp,
                    vt_sb[ds(D * head, D), ts(kt32, P)],
                    ident[ds(D * head, D), ds(D * head, D)],
                )
                nc.vector.tensor_copy(vp_sb[:, head, kt32, 0:D], tp)

            # PT ring: 2 heads x 32 slots x [128, 1024] fp16
            RING = 32
            pt_all = ptpool.tile([P, 2, RING, 1024], f16, tag="pt")

            ot_tiles = {}

            def pv_qc(unit, head, qc, k0, k1, ctx=None):
                """P@V for one 128-q chunk: ctx[q,65] += PT[k,qc].T @ V'."""
                b = unit // 2
                if ctx is None:
                    ctx = psp.tile([P, D + 1], f32, tag="ctx", bufs=2, name="ctx")
                for kt in range(k0, k1):
                    nc.tensor.matmul(
                        ctx,
                        pt_all[:, head, (unit * 16 + kt) % RING, ts(qc, P)],
                        vp_sb[:, head, b * 16 + kt, :],
                        start=(kt == 0),
                        stop=(kt == 15),
                    )
                return ctx

            def pv_norm(ctx, unit, head, qc):
                """Normalize one ctx chunk into the (unit, head) staging tile;
                DMA the full 1024 q rows out after the last chunk."""
                qbase = (unit // 2) * S + (unit % 2) * 1024
                key = (unit, head)
                if key not in ot_tiles:
                    ot_tiles[key] = otpool.tile([P, 8, D], f32, tag="ot", name="ot")
                ot = ot_tiles[key]
                rc = smallpool.tile([P, 1], f32, tag="rc")
                nc.vector.reciprocal(rc, ctx[:, D : D + 1])
                nc.vector.tensor_scalar_mul(ot[:, qc, :], ctx[:, 0:D], rc)
                if qc in (3, 7):
                    half = qc // 4
                    dst = out[
                        ds(qbase + half * 512, 512), ds(D * head, D)
                    ].rearrange("(qc p) d -> p qc d", p=P)
                    nc.sync.dma_start(dst, ot[:, ds(half * 4, 4), :])

            def pv_full(unit, head, qc):
                ctx = pv_qc(unit, head, qc, 0, 16)
                pv_norm(ctx, unit, head, qc)

            # Deferred-work queue: (cost, fn, deadline). Deadline (u, kt)
            # means the item MUST be emitted before (u, kt)'s scores/exp.
            work_q = deque()

            def q_proj(t8, which, dl):
                work_q.append((1.7, lambda: proj_group(t8, which), dl))

            def q_vp4(b, group, dl):  # 4 k-tiles x 2 heads
                for kk in range(4 * group, 4 * group + 4):
                    for head in range(2):
                        work_q.append(
                            (0.15, lambda h=head, k=kk: vprime(h, b * 16 + k), dl)
                        )

            def q_pv(unit, head, qcs, dl):
                for qc in qcs:
                    work_q.append(
                        (0.45, lambda h=head, q=qc: pv_full(unit, h, q), dl)
                    )

            NEVER = (9, 0)

            def push_unit_work(unit):
                if unit == 0:
                    q_proj(1, 1, (0, 4))  # k1
                    q_proj(0, 2, (1, 0))  # v0 (feeds V' -> pv(0) in unit 1)
                    q_vp4(0, 0, (1, 0))
                    q_proj(1, 2, (1, 0))
                    q_vp4(0, 1, (1, 0))
                    q_proj(2, 1, (0, 8))  # k2
                    q_proj(2, 2, (1, 0))
                    q_proj(2, 0, (1, 0))  # q2 (unit 1 scores)
                    q_vp4(0, 2, (1, 0))
                    q_proj(3, 1, (0, 12))  # k3
                    q_proj(3, 0, (1, 0))  # q3
                    q_proj(3, 2, (1, 0))
                    q_vp4(0, 3, (1, 0))
                elif unit == 1:
                    q_pv(0, 0, range(0, 4), (2, 0))
                    q_proj(4, 1, (2, 0))  # k4
                    q_pv(0, 0, range(4, 8), (2, 0))
                    q_proj(4, 0, (2, 0))  # q4
                    q_pv(0, 1, range(0, 4), (2, 0))
                    q_proj(5, 0, (2, 0))  # q5
                    q_pv(0, 1, range(4, 8), (2, 0))
                    q_proj(4, 2, (3, 0))  # v4 + V' feed pv(2) in unit 3
                    q_vp4(1, 0, (3, 0))
                elif unit == 2:
                    q_proj(5, 1, (2, 4))  # k5
                    q_pv(1, 0, range(0, 4), (3, 0))
                    q_proj(6, 1, (2, 8))  # k6
                    q_pv(1, 0, range(4, 8), (3, 0))
                    q_proj(7, 1, (2, 12))  # k7
                    q_proj(6, 0, (3, 0))  # q6
                    q_pv(1, 1, range(0, 4), (3, 0))
                    q_proj(7, 0, (3, 0))  # q7
                    q_pv(1, 1, range(4, 8), (3, 0))
                elif unit == 3:
                    # batch 1's remaining V' (FIFO keeps them ahead of pv(2))
                    q_proj(5, 2, NEVER)
                    q_vp4(1, 1, NEVER)
                    q_proj(6, 2, NEVER)
                    q_vp4(1, 2, NEVER)
                    q_proj(7, 2, NEVER)
                    q_vp4(1, 3, NEVER)
                    q_pv(2, 0, range(8), NEVER)
                    q_pv(2, 1, range(8), NEVER)

            # ---- batch 0 essentials: just enough for unit 0's scores ----
            proj_group(0, 0)  # q0
            proj_group(1, 0)  # q1
            proj_group(0, 1)  # k0

            for unit in range(4):
                b, qh = unit // 2, unit % 2
                base = b * S
                qbase = base + qh * 1024
                if unit == 1:
                    # drain every batch-0 consumer of xt_sb first (emission
                    # order is semantic order), then reload X^T with batch 1
                    while work_q and work_q[0][2] <= (1, 0):
                        work_q.popleft()[1]()
                    for quarter in range(4):
                        nc.sync.dma_start(
                            xt_sb[:, :, ts(quarter, 512)],
                            xtp[:, :, ds(S + quarter * 512, 512)],
                        )
                push_unit_work(unit)
                credit = 2.0
                for kt in range(16):
                    while work_q and work_q[0][2] <= (unit, kt):
                        _, fn, _ = work_q.popleft()
                        fn()
                    sts = []
                    for head in range(2):
                        st = psp.tile(
                            [P, 1024], f32, tag="st", bufs=2, name=f"st{head}"
                        )
                        sts.append(st)
                    for j in range(2):
                        for head in range(2):
                            hb = D * head
                            nc.tensor.matmul(
                                sts[head][:, ts(j, 512)],
                                kt_sb[ds(hb, D), ds(base + kt * P, P)],
                                qt_sb[ds(hb, D), ds(qbase + j * 512, 512)],
                                start=True,
                                stop=True,
                            )
                    slot = (unit * 16 + kt) % RING
                    for head in range(2):
                        dstp = pt_all[:, head, slot, :]
                        if (kt, head) in DVE_EXP[unit]:
                            nc.vector.tensor_scalar(
                                dstp.bitcast(u16),
                                sts[head],
                                A16,
                                B16,
                                mybir.AluOpType.mult,
                                mybir.AluOpType.add,
                            )
                        else:
                            nc.scalar.activation(dstp, sts[head], Exp, scale=0.125)
                    # deferred work drained under the kt shadow, paced
                    credit = min(credit + 1.3, 8.0)
                    while work_q and work_q[0][0] <= credit:
                        cost, fn, _ = work_q.popleft()
                        credit -= cost
                        fn()
            while work_q:
                work_q.popleft()[1]()
            # unit 3's P@V has no later exp shadow: straight-line tail,
            # heads interleaved so norms/DMAs overlap remaining matmuls
            for qc in range(8):
                for head in range(2):
                    pv_full(3, head, qc)

    nc.compile()
    return nc


def get_nc():
    if "nc" not in _CACHED:
        _CACHED["nc"] = _build()
    return _CACHED["nc"]


def kernel(hidden_states, Wq, bq, Wk, bk, Wv, bv):
    from concourse.bass_utils import run_bass_kernel_spmd

    nc = get_nc()

    x2 = np.asarray(hidden_states, dtype=np.float32).reshape(T, HID)
    xt_b = np.ascontiguousarray(x2.T).astype(F16)

    in_maps = []
    for c in range(N_CORES):
        sl = slice(P * c, P * (c + 1))
        in_maps.append(
            {
                "xt": xt_b,
                "wq": np.ascontiguousarray(np.asarray(Wq, np.float32)[:, sl]).astype(F16),
                "wk": np.ascontiguousarray(np.asarray(Wk, np.float32)[:, sl]).astype(F16),
                "wv": np.ascontiguousarray(np.asarray(Wv, np.float32)[:, sl]).astype(F16),
                "bias": np.ascontiguousarray(
                    np.stack(
                        [
                            np.asarray(bq, np.float32)[sl],
                            np.asarray(bk, np.float32)[sl],
                            np.asarray(bv, np.float32)[sl],
                        ],
                        axis=1,
                    )
                ),
            }
        )

    res = run_bass_kernel_spmd(nc, in_maps, list(range(N_CORES)))

    full = np.empty((T, HID), dtype=np.float32)
    for c in range(N_CORES):
        full[:, P * c : P * (c + 1)] = res.results[c]["out"]
    return full.reshape(B, S, HID)

